# revision 1
# baseline (speedup 1.0000x reference)
"""2-layer GCN (GCNConv x2) on 8 trn2 NeuronCores.

Strategy (node/graph parallel, per sharding hint):
  - Nodes are ranked by in-degree (desc) and dealt round-robin to the 8
    cores in strata of 1024 ranks (128 nodes/core/stratum) so that every
    core's block b has a near-identical max in-degree -> uniform gather
    width kb[b] across cores -> one SPMD program for all 8 cores.
  - norm(e) = dinv[src]*dinv[dst] factorizes. Layer 1 exploits that the
    GEMM commutes with aggregation: out1 = relu(dinv_d*(sum dinv_s x_s)W1)
    so the device gathers rows of a host-prebuilt table XS = dinv*x and
    runs ONE [128,64]@[64,64] GEMM per block after the reduce — no
    x@W1 prologue and no H1 table at all.
  - Indirect gathers carry ONE offset per partition per instruction (HW
    limit of the INDIRECT1D path: each offset moves the out AP's whole
    per-channel contiguous run; out APs must be 2-dim [128, F]; tables
    must be Internal DRAM — inputs are staged XS->XSI through SBUF).
    ~2000 gather instructions at ~1.4us/instr of Pool-serial SWDGE
    generation are the dominant cost.
  - Self-loop columns avoid the Pool queue entirely: L1 reads a per-core
    self-row appendix of the table (rows [TAB, XTAB)) via direct DMA at
    a core-independent address; L2 reads the core-local H2P part.
  - Per block the slot dim is reduced with a single DVE tensor_reduce
    over a strided transposed view; scale/relu are fused on Act.
  - Layer 2: h~2 = dinv*(relu@W2) parts are AllGather'd (chunk-major
    layout, 4 chunks overlapped under the L1 phase); the L2 gather reads
    the gathered table by per-edge row index.
"""

import numpy as np

N = 50000
E = 1000000
F_IN, F_HID, F_OUT = 64, 64, 32
P = 128
NCORES = 8
STR = P * NCORES           # 1024 ranks per stratum
NB = (N + STR - 1) // STR  # 49 blocks per core
NPAD = NB * STR            # 50176 padded node count
TAB = NPAD + P             # XS table rows; rows [NPAD, TAB) are zeros
ZROW = NPAD                # a guaranteed-zero row (both tables)
LOCN = NB * P              # 6272 nodes per core
XTAB = TAB + LOCN          # XSI rows: table + per-core self-row appendix
CMAX = 120                 # max gather columns per block group (tile width)
GSTEP = 1                  # gather columns per indirect DMA instruction
                           # (HW limit: 1 offset per partition per instruction)
QBLOCKS = [14, 13, 12, 10] # collective chunk sizes (blocks), sum = NB

_last_results = None       # stash for test.py introspection
_nc_cache = {}             # build-key -> compiled Bass program


def _make_groups(kb):
    """Greedy-pack consecutive blocks into gather groups of <= CMAX cols."""
    groups = []  # (b0, nblocks, cols)
    b = 0
    while b < NB:
        cols = kb[b] + 1
        nb_g = 1
        while b + nb_g < NB and cols + kb[b + nb_g] + 1 <= CMAX:
            cols += kb[b + nb_g] + 1
            nb_g += 1
        groups.append((b, nb_g, cols))
        b += nb_g
    return groups


def _host_prep(x, edge_index, W1, b1, W2, b2):
    import ml_dtypes

    src = np.asarray(edge_index[0], dtype=np.int64)
    dst = np.asarray(edge_index[1], dtype=np.int64)
    x = np.asarray(x, dtype=np.float32)

    deg = np.bincount(dst, minlength=N).astype(np.int64) + 1  # incl self-loop
    dinv = (1.0 / np.sqrt(deg.astype(np.float64))).astype(np.float32)

    node_perm = np.argsort(-deg, kind="stable")      # rank -> node
    rank = np.empty(N, dtype=np.int64)
    rank[node_perm] = np.arange(N)

    # rank -> (core, block, pos); local row on core = block*P + pos
    r_s = rank[src]
    r_d = rank[dst]
    c_d = (r_d % STR) % NCORES
    b_d = r_d // STR
    p_d = (r_d % STR) // NCORES
    c_s = (r_s % STR) % NCORES
    b_s = r_s // STR
    p_s = (r_s % STR) // NCORES

    # per-(core,slot) edge position j
    slot = b_d * P + p_d
    key = c_d * LOCN + slot
    order_e = np.argsort(key, kind="stable")
    ks = key[order_e]
    starts = np.searchsorted(ks, np.arange(NCORES * LOCN))
    cum = np.arange(len(ks), dtype=np.int64) - starts[ks]
    j = np.empty(len(ks), dtype=np.int64)
    j[order_e] = cum

    cnt = np.bincount(key, minlength=NCORES * LOCN)
    kb = cnt.reshape(NCORES, NB, P).max(axis=(0, 2)).astype(np.int64)
    kb = np.maximum(kb, 1)
    kb_l = [int(v) for v in kb]
    groups = _make_groups(kb_l)

    # column base per block: groups are consecutive; within a group each
    # block b owns cols [colbase[b], colbase[b]+kb[b]+1), col 0 = self.
    colbase = np.zeros(NB, dtype=np.int64)
    goff = np.zeros(len(groups), dtype=np.int64)     # group col offsets (total)
    gcols = np.zeros(NB, dtype=np.int64)             # cols of owning group
    tot = 0
    for gi, (b0, nb_g, cols) in enumerate(groups):
        goff[gi] = tot
        cb = 0
        for b in range(b0, b0 + nb_g):
            colbase[b] = cb
            gcols[b] = cols
            cb += kb_l[b] + 1
        tot += cols
    TOTC = tot
    grp_of = np.zeros(NB, dtype=np.int64)
    for gi, (b0, nb_g, _) in enumerate(groups):
        grp_of[b0:b0 + nb_g] = gi

    # chunk-major H2 layout for the chunked AllGather
    qb0 = np.cumsum([0] + QBLOCKS)                   # chunk block starts
    chunk_of = np.zeros(NB, dtype=np.int64)
    for q in range(len(QBLOCKS)):
        chunk_of[qb0[q]:qb0[q + 1]] = q
    qbase = np.zeros(len(QBLOCKS), dtype=np.int64)   # H2 row base per chunk
    acc = 0
    for q in range(len(QBLOCKS)):
        qbase[q] = acc
        acc += NCORES * QBLOCKS[q] * P
    assert acc == NPAD

    qblocks_a = np.asarray(QBLOCKS, dtype=np.int64)

    def h2row(c, b, p):
        q = chunk_of[b]
        return qbase[q] + c * qblocks_a[q] * P + (b - qb0[q]) * P + p

    # S1/S2 flat layout per core: concat over groups of [P, cols_g] p-major
    # flat = goff[g]*P + p*cols_g + (colbase[b] + col_in_block)
    S1 = np.full((NCORES, TOTC * P), ZROW, dtype=np.int32)
    S2 = np.full((NCORES, TOTC * P), ZROW, dtype=np.int32)

    # edges
    flat_e = (goff[grp_of[b_d]] * P + p_d * gcols[b_d]
              + colbase[b_d] + 1 + j)
    S1[c_d, flat_e] = r_s.astype(np.int32)
    S2[c_d, flat_e] = h2row(c_s, b_s, p_s).astype(np.int32)

    # self cols (col 0 of each block)
    bs, ps_ = np.meshgrid(np.arange(NB), np.arange(P), indexing="ij")
    flat_s = goff[grp_of[bs]] * P + ps_ * gcols[bs] + colbase[bs]
    for c in range(NCORES):
        selfr = bs * STR + ps_ * NCORES + c            # rank of own node
        S1[c, flat_s] = selfr.astype(np.int32)
        S2[c, flat_s] = h2row(c, bs, ps_).astype(np.int32)

    # dinv in rank order (ghost ranks >= N keep 1.0; their inputs are 0)
    dinv_r = np.ones(NPAD, dtype=np.float32)
    dinv_r[rank] = dinv
    dinv_B = dinv_r.reshape(NB, P, NCORES).transpose(2, 1, 0).copy()  # [c][P, NB]

    # XS table: rank-ordered dinv*x, bf16, zero-padded; per core append its
    # own self rows at [TAB, XTAB) so L1 self-columns use a direct DMA from
    # a core-independent address (the SPMD program is shared across cores)
    XS = np.zeros((TAB, F_IN), dtype=np.float32)
    XS[rank] = x * dinv[:, None]
    XS = XS.astype(ml_dtypes.bfloat16)
    bs_f = np.repeat(np.arange(NB), P)
    ps_f = np.tile(np.arange(P), NB)
    XS_cores = []
    for c in range(NCORES):
        selfranks = bs_f * STR + ps_f * NCORES + c
        XS_cores.append(np.ascontiguousarray(
            np.vstack([XS, XS[selfranks]])))

    W1b = np.asarray(W1, np.float32).astype(ml_dtypes.bfloat16)
    W2b = np.asarray(W2, np.float32).astype(ml_dtypes.bfloat16)
    b1f = np.asarray(b1, np.float32)
    b2f = np.asarray(b2, np.float32)
    has_b1 = bool(np.any(b1f))
    has_b2 = bool(np.any(b2f))

    in_maps = []
    for c in range(NCORES):
        m = {
            "XS": XS_cores[c], "W1": W1b, "W2": W2b,
            "DB": np.ascontiguousarray(dinv_B[c]),
            "S1": S1[c], "S2": S2[c],
        }
        if has_b1:
            m["B1"] = np.ascontiguousarray(
                np.broadcast_to(b1f, (P, F_HID)))
        if has_b2:
            m["B2"] = np.ascontiguousarray(
                np.broadcast_to(b2f, (P, F_OUT)))
        in_maps.append(m)
    return in_maps, kb_l, groups, has_b1, has_b2, node_perm


def _build(kb, groups, has_b1, has_b2, debug=False):
    from contextlib import ExitStack
    import concourse.bass as bass
    import concourse.tile as tile
    from concourse import bacc, mybir
    from concourse.masks import make_identity

    dt = mybir.dt
    AFT = mybir.ActivationFunctionType
    AX = mybir.AxisListType
    ALU = mybir.AluOpType
    TOTC = sum(c for _, _, c in groups)
    qb0 = np.cumsum([0] + QBLOCKS)
    qbase = np.cumsum([0] + [NCORES * q * P for q in QBLOCKS])

    nc = bacc.Bacc("TRN2", target_bir_lowering=False, debug=False,
                   num_devices=NCORES)

    XS = nc.dram_tensor("XS", [XTAB, F_IN], dt.bfloat16, kind="ExternalInput").ap()
    W1 = nc.dram_tensor("W1", [F_IN, F_HID], dt.bfloat16, kind="ExternalInput").ap()
    W2 = nc.dram_tensor("W2", [F_HID, F_OUT], dt.bfloat16, kind="ExternalInput").ap()
    DB = nc.dram_tensor("DB", [P, NB], dt.float32, kind="ExternalInput").ap()
    S1 = nc.dram_tensor("S1", [TOTC * P], dt.int32, kind="ExternalInput").ap()
    S2 = nc.dram_tensor("S2", [TOTC * P], dt.int32, kind="ExternalInput").ap()
    if has_b1:
        B1 = nc.dram_tensor("B1", [P, F_HID], dt.float32, kind="ExternalInput").ap()
    if has_b2:
        B2 = nc.dram_tensor("B2", [P, F_OUT], dt.float32, kind="ExternalInput").ap()
    OUT = nc.dram_tensor("OUT", [LOCN, F_OUT], dt.float32, kind="ExternalOutput").ap()
    H2P = nc.dram_tensor("H2P", [LOCN, F_OUT], dt.bfloat16, kind="Internal").ap()
    H2 = nc.dram_tensor("H2", [TAB, F_OUT], dt.bfloat16, kind="Internal").ap()
    # indirect gathers must read an Internal table: input-tensor base
    # addresses are runtime-assigned and do not resolve via the indirect path
    XSI = nc.dram_tensor("XSI", [XTAB, F_IN], dt.bfloat16, kind="Internal").ap()
    if debug:
        cols0 = groups[0][2]
        DBG_XSI = nc.dram_tensor("DBG_XSI", [P, F_IN], dt.bfloat16,
                                 kind="ExternalOutput").ap()
        DBG_G1 = nc.dram_tensor("DBG_G1", [P, cols0 * F_HID], dt.bfloat16,
                                kind="ExternalOutput").ap()
        DBG_AGG = nc.dram_tensor("DBG_AGG", [P, F_HID], dt.float32,
                                 kind="ExternalOutput").ap()
        DBG_O1 = nc.dram_tensor("DBG_O1", [P, F_HID], dt.bfloat16,
                                kind="ExternalOutput").ap()

    with ExitStack() as ctx:
        tc = ctx.enter_context(tile.TileContext(nc))
        const = ctx.enter_context(tc.tile_pool(name="const", bufs=1))
        w1s = const.tile([F_IN, F_HID], dt.bfloat16)
        nc.sync.dma_start(w1s[:], W1)
        w2s = const.tile([F_HID, F_OUT], dt.bfloat16)
        nc.sync.dma_start(w2s[:], W2)
        dbs = const.tile([P, NB], dt.float32)
        nc.sync.dma_start(dbs[:], DB)
        if has_b1:
            b1s = const.tile([P, F_HID], dt.float32)
            nc.sync.dma_start(b1s[:], B1)
        if has_b2:
            b2s = const.tile([P, F_OUT], dt.float32)
            nc.sync.dma_start(b2s[:], B2)
        ident = const.tile([P, P], dt.bfloat16)
        make_identity(nc, ident[:])
        zt = const.tile([P, F_OUT], dt.bfloat16)
        nc.gpsimd.memset(zt[:], 0.0)
        nc.sync.dma_start(H2[NPAD:TAB, :], zt[:])

        # stage XS (ExternalInput) -> XSI (Internal) through SBUF
        FLATN = XTAB * F_IN // P         # 28288 elems/partition
        NSTG = 4
        SGC = FLATN // NSTG              # 7072
        xs_v = XS.rearrange("(p x) f -> p (x f)", p=P)
        xsi_v = XSI.rearrange("(p x) f -> p (x f)", p=P)
        stg = ctx.enter_context(tc.tile_pool(name="stg", bufs=2))
        for s in range(NSTG):
            st = stg.tile([P, SGC], dt.bfloat16)
            nc.sync.dma_start(st[:], xs_v[:, s * SGC:(s + 1) * SGC])
            nc.scalar.dma_start(xsi_v[:, s * SGC:(s + 1) * SGC], st[:])
        if debug:
            dxt = stg.tile([P, F_IN], dt.bfloat16)
            nc.sync.dma_start(dxt[:], XSI[0:P, :])
            nc.sync.dma_start(DBG_XSI, dxt[:])

        ipool = ctx.enter_context(tc.tile_pool(name="idx", bufs=8))
        g1pool = ctx.enter_context(tc.tile_pool(name="g1", bufs=4))
        g2pool = ctx.enter_context(tc.tile_pool(name="g2", bufs=4))
        apool = ctx.enter_context(tc.tile_pool(name="agg", bufs=6))
        opool = ctx.enter_context(tc.tile_pool(name="o", bufs=6))
        h2pool = ctx.enter_context(tc.tile_pool(name="h2", bufs=4))
        pst_p = ctx.enter_context(tc.tile_pool(name="pst", bufs=2, space="PSUM"))
        psm_p = ctx.enter_context(tc.tile_pool(name="psm", bufs=2, space="PSUM"))

        # ---- Layer 1: gather XS, reduce, GEMM W1, relu, GEMM W2 -> H2P ----
        pending_chunks = []   # chunks whose collective is not yet issued
        done_chunk = 0
        off = 0
        for gi, (b0, nb_g, cols) in enumerate(groups):
            # issue collectives for chunks finished >= 1 full group ago
            # (slack keeps the Pool queue from stalling on H2P write sems)
            while (pending_chunks and gi >= 1
                   and pending_chunks[0][1] < groups[gi - 1][0]):
                q, _ = pending_chunks.pop(0)
                nq = QBLOCKS[q]
                nc.gpsimd.collective_compute(
                    "AllGather", mybir.AluOpType.bypass,
                    replica_groups=[list(range(NCORES))],
                    ins=[H2P[qb0[q] * P:qb0[q + 1] * P, :]],
                    outs=[H2[qbase[q]:qbase[q] + NCORES * nq * P, :]],
                )
            idx1 = ipool.tile([P, cols], dt.int32)
            nc.sync.dma_start(
                idx1[:], S1[off * P:(off + cols) * P].rearrange(
                    "(p k) -> p k", p=P))
            G1 = g1pool.tile([P, cols, F_HID], dt.bfloat16)
            # self columns via direct DMA from the per-core appendix rows
            selfcols1 = {}
            cb = 0
            for b in range(b0, b0 + nb_g):
                selfcols1[cb] = b
                cb += kb[b] + 1
            for c0 in range(0, cols, GSTEP):
                cc = min(GSTEP, cols - c0)
                if cc == 1 and c0 in selfcols1:
                    sb = selfcols1[c0]
                    nc.scalar.dma_start(
                        G1[:, c0, :],
                        XSI[TAB + sb * P:TAB + (sb + 1) * P, :])
                    continue
                if cc == 1:
                    gout = G1[:, c0, :]
                else:
                    gout = G1[:, c0:c0 + cc, :].rearrange("p c f -> p (c f)")
                nc.gpsimd.indirect_dma_start(
                    out=gout, out_offset=None, in_=XSI,
                    in_offset=bass.IndirectOffsetOnAxis(
                        ap=idx1[:, c0:c0 + cc], axis=0),
                )
            if debug and gi == 0:
                nc.sync.dma_start(
                    DBG_G1, G1[:].rearrange("p c f -> p (c f)"))
            coff = 0
            for b in range(b0, b0 + nb_g):
                k = kb[b] + 1
                agg = apool.tile([P, F_HID], dt.float32)
                nc.vector.tensor_reduce(
                    agg[:], G1[:, coff:coff + k, :].transpose([0, 2, 1]),
                    axis=AX.X, op=ALU.add)
                if debug and b == b0 == 0:
                    nc.sync.dma_start(DBG_AGG, agg[:])
                o0 = opool.tile([P, F_HID], dt.bfloat16)
                nc.scalar.activation(o0[:], agg[:], AFT.Copy,
                                     scale=dbs[:, b:b + 1])
                pst1 = pst_p.tile([F_HID, P], dt.bfloat16, space="PSUM")
                nc.tensor.transpose(pst1[:], o0[:], ident[:])
                o0T = opool.tile([F_HID, P], dt.bfloat16)
                nc.scalar.activation(o0T[:], pst1[:], AFT.Copy)
                ps1 = psm_p.tile([P, F_HID], dt.float32, space="PSUM")
                nc.tensor.matmul(ps1[:], lhsT=o0T[:], rhs=w1s[:],
                                 start=True, stop=True)
                o1 = opool.tile([P, F_HID], dt.bfloat16)
                if has_b1:
                    t = apool.tile([P, F_HID], dt.float32)
                    nc.vector.tensor_add(t[:], ps1[:], b1s[:])
                    nc.scalar.activation(o1[:], t[:], AFT.Relu)
                else:
                    nc.scalar.activation(o1[:], ps1[:], AFT.Relu)
                if debug and b == 0:
                    nc.sync.dma_start(DBG_O1, o1[:])
                pst2 = pst_p.tile([F_HID, P], dt.bfloat16, space="PSUM")
                nc.tensor.transpose(pst2[:], o1[:], ident[:])
                o1T = opool.tile([F_HID, P], dt.bfloat16)
                nc.scalar.activation(o1T[:], pst2[:], AFT.Copy)
                ps2 = psm_p.tile([P, F_OUT], dt.float32, space="PSUM")
                nc.tensor.matmul(ps2[:], lhsT=o1T[:], rhs=w2s[:],
                                 start=True, stop=True)
                h2s = h2pool.tile([P, F_OUT], dt.bfloat16)
                nc.scalar.activation(h2s[:], ps2[:], AFT.Copy,
                                     scale=dbs[:, b:b + 1])
                nc.scalar.dma_start(H2P[b * P:(b + 1) * P, :], h2s[:])
                coff += k
                if (done_chunk < len(QBLOCKS)
                        and b + 1 == qb0[done_chunk + 1]):
                    pending_chunks.append((done_chunk, b))
                    done_chunk += 1
            off += cols

        for q, _ in pending_chunks:
            nq = QBLOCKS[q]
            nc.gpsimd.collective_compute(
                "AllGather", mybir.AluOpType.bypass,
                replica_groups=[list(range(NCORES))],
                ins=[H2P[qb0[q] * P:qb0[q + 1] * P, :]],
                outs=[H2[qbase[q]:qbase[q] + NCORES * nq * P, :]],
            )

        # ---- Layer 2: gather H2 (self col included), reduce, scale ----
        off = 0
        for gi, (b0, nb_g, cols) in enumerate(groups):
            idx2 = ipool.tile([P, cols], dt.int32)
            nc.sync.dma_start(
                idx2[:], S2[off * P:(off + cols) * P].rearrange(
                    "(p k) -> p k", p=P))
            G2 = g2pool.tile([P, cols, F_OUT], dt.bfloat16)
            # self columns come from the core-local H2P part via direct DMA
            # (off the Pool queue; prefetchable while the AllGather runs)
            selfcols = {}
            cb = 0
            for b in range(b0, b0 + nb_g):
                selfcols[cb] = b
                cb += kb[b] + 1
            for c0 in range(0, cols, GSTEP):
                cc = min(GSTEP, cols - c0)
                if cc == 1 and c0 in selfcols:
                    sb = selfcols[c0]
                    nc.scalar.dma_start(G2[:, c0, :],
                                        H2P[sb * P:(sb + 1) * P, :])
                    continue
                if cc == 1:
                    gout = G2[:, c0, :]
                else:
                    gout = G2[:, c0:c0 + cc, :].rearrange("p c f -> p (c f)")
                nc.gpsimd.indirect_dma_start(
                    out=gout, out_offset=None, in_=H2,
                    in_offset=bass.IndirectOffsetOnAxis(
                        ap=idx2[:, c0:c0 + cc], axis=0),
                )
            coff = 0
            for b in range(b0, b0 + nb_g):
                k = kb[b] + 1
                agg2 = apool.tile([P, F_OUT], dt.float32)
                nc.vector.tensor_reduce(
                    agg2[:], G2[:, coff:coff + k, :].transpose([0, 2, 1]),
                    axis=AX.X, op=ALU.add)
                ot = opool.tile([P, F_OUT], dt.float32)
                nc.scalar.activation(ot[:], agg2[:], AFT.Copy,
                                     scale=dbs[:, b:b + 1])
                if has_b2:
                    ot2 = opool.tile([P, F_OUT], dt.float32)
                    nc.vector.tensor_add(ot2[:], ot[:], b2s[:])
                    ot = ot2
                nc.sync.dma_start(OUT[b * P:(b + 1) * P, :], ot[:])
                coff += k
            off += cols

    nc.compile()
    return nc


def _ensure_ntff_hook():
    """Install the axon NTFF profile hook if the antenv stub lacks it."""
    import sys
    import types
    try:
        from antenv.axon_hooks import get_axon_ntff_profile_hook  # noqa: F401
        return
    except ImportError:
        pass
    try:
        import antenv
        from trn_agent_boot.trn_boot import _ntff_profile_via_ctypes
        hook = _ntff_profile_via_ctypes("/opt/axon/libaxon_pjrt.so")
        mod = types.ModuleType("antenv.axon_hooks")
        mod._hook = hook
        mod.get_axon_ntff_profile_hook = lambda: mod._hook
        mod.set_axon_ntff_profile_hook = lambda h: setattr(mod, "_hook", h)
        sys.modules["antenv.axon_hooks"] = mod
        antenv.axon_hooks = mod
    except Exception as e:  # tracing is best-effort
        print(f"ntff hook install failed: {e}")


def kernel(x, edge_index, W1, b1, W2, b2, _trace=False, _sim=False,
           _debug=False):
    global _last_results
    from concourse.bass_utils import run_bass_kernel_spmd
    if _trace:
        _ensure_ntff_hook()

    in_maps, kb, groups, has_b1, has_b2, node_perm = _host_prep(
        x, edge_index, W1, b1, W2, b2)
    key = (tuple(kb), has_b1, has_b2, _debug)
    nc = _nc_cache.get(key)
    if nc is None:
        nc = _nc_cache[key] = _build(kb, groups, has_b1, has_b2,
                                     debug=_debug)

    if _sim:
        from concourse.bass_interp import MultiCoreSim
        sim = MultiCoreSim(nc, num_cores=NCORES)
        cores = [sim.cores[i] for i in range(NCORES)]
        for c, core in enumerate(cores):
            for name, arr in in_maps[c].items():
                core.tensor(name)[:] = arr
        sim.simulate(check_with_hw=False)
        parts = [np.array(core.tensor("OUT")) for core in cores]
    else:
        res = run_bass_kernel_spmd(
            nc, in_maps, core_ids=list(range(NCORES)), trace=_trace)
        _last_results = res
        parts = [r["OUT"] for r in res.results]

    # unshard: core c, local row b*P+p -> rank b*STR + p*NCORES + c
    out = np.empty((N, F_OUT), dtype=np.float32)
    allp = np.stack(parts)                          # [c, LOCN, F_OUT]
    allp = allp.reshape(NCORES, NB, P, F_OUT)       # [c, b, p, f]
    by_rank = allp.transpose(1, 2, 0, 3).reshape(NPAD, F_OUT)  # rank-major
    out[node_perm] = by_rank[:N]
    return out

